# revision 8
# baseline (speedup 1.0000x reference)
"""AverageFusion kernel for 8 TRN2 NeuronCores.

ref semantics (per patient row b):
    fused[b, :]  = x_clinical[b] + x_mRNA[b] + x_miRNA[b] + x_CNV[b]
    zero_dims[b] = #{m : sum_l x_m[b, l] == 0.0}
    out[b, :]    = fused[b, :] * (zero_dims[b] + 1)     # note: z+1 == where(z>0, z+1, 1)

Pure data parallel: each core owns B/8 = 2048 rows. No collectives.
"""

import numpy as np

import concourse.bass as bass
import concourse.bacc as bacc
import concourse.mybir as mybir
from concourse.tile import TileContext
from concourse.bass_utils import run_bass_kernel_spmd

B, L = 16384, 1024
N_CORES = 8
B_SHARD = B // N_CORES          # 2048 rows per core
P = 128                         # SBUF partitions
N_CHUNKS = B_SHARD // P         # 16 chunks of [128, L] per core

NAMES = ["x_clinical", "x_mRNA", "x_miRNA", "x_CNV"]
F32 = mybir.dt.float32
ALU = mybir.AluOpType
AF = mybir.ActivationFunctionType


def build_bass(repeat=1):
    nc = bacc.Bacc()
    xs = [nc.declare_dram_parameter(n, [B_SHARD, L], F32, isOutput=False) for n in NAMES]
    out_ext = nc.declare_dram_parameter("out", [B_SHARD, L], F32, isOutput=True)

    with TileContext(nc) as tc:
        with (
            tc.tile_pool(name="io", bufs=3) as io_pool,
            tc.tile_pool(name="fusedp", bufs=3) as fused_pool,
            tc.tile_pool(name="scratch", bufs=2) as scratch_pool,
            tc.tile_pool(name="small", bufs=4) as small_pool,
        ):

            def body(_iv=None):
                _build_chunks(nc, tc, xs, out_ext, io_pool, fused_pool,
                              scratch_pool, small_pool)

            if repeat == 1:
                body()
            else:
                with tc.For_i(0, repeat, 1):
                    body()

    nc.compile()
    return nc


def _build_chunks(nc, tc, xs, out_ext, io_pool, fused_pool, scratch_pool, small_pool):
    if True:
        if True:
            for c in range(N_CHUNKS):
                r0 = c * P
                xt = []
                for m in range(4):
                    t = io_pool.tile([P, L], F32, tag=f"in{m}")
                    nc.sync.dma_start(out=t[:], in_=xs[m][r0 : r0 + P, :])
                    xt.append(t)

                rt = small_pool.tile([P, 4], F32, tag="rsums")
                junk = small_pool.tile([P, 4], F32, tag="junk")
                scaler = small_pool.tile([P, 1], F32, tag="scaler")
                scratch = scratch_pool.tile([P, L], F32, tag="scratch")
                fused = fused_pool.tile([P, L], F32, tag="fused")

                # ScalarE (ACT): row sums via activation accum.  m=0's copy
                # doubles as the seed of the fused accumulator, so every DVE
                # add below waits on exactly one DMA queue semaphore.
                nc.scalar.activation(
                    out=fused[:], in_=xt[0][:], func=AF.Copy,
                    accum_out=rt[:, 0:1],
                )
                for m in (1, 2):
                    nc.scalar.activation(
                        out=scratch[:], in_=xt[m][:], func=AF.Copy,
                        accum_out=rt[:, m : m + 1],
                    )

                # VectorE (DVE): fused += x1, x2, x3
                nc.vector.tensor_add(out=fused[:], in0=fused[:], in1=xt[1][:])
                nc.vector.tensor_add(out=fused[:], in0=fused[:], in1=xt[2][:])
                nc.vector.tensor_add(out=fused[:], in0=fused[:], in1=xt[3][:])
                # row sum of modality 3 on DVE
                nc.vector.reduce_sum(
                    out=rt[:, 3:4], in_=xt[3][:], axis=mybir.AxisListType.X
                )
                # scaler = 1 + #zeros:  accum = (Σ (r == 0)) + 1 = zero_dims + 1
                nc.vector.tensor_scalar(
                    out=junk[:],
                    in0=rt[:],
                    scalar1=0.0,
                    scalar2=1.0,
                    op0=ALU.is_equal,
                    op1=ALU.add,
                    accum_out=scaler[:, 0:1],
                )
                # out = fused * scaler (per-partition scalar)
                nc.vector.tensor_scalar_mul(fused[:], fused[:], scaler[:, 0:1])
                nc.sync.dma_start(out=out_ext[r0 : r0 + P, :], in_=fused[:])


def _run(in_maps, **kwargs):
    nc = build_bass()
    return run_bass_kernel_spmd(nc, in_maps, core_ids=list(range(N_CORES)), **kwargs)


def _shard(full):
    full = np.ascontiguousarray(np.asarray(full, dtype=np.float32))
    assert full.shape == (B, L), full.shape
    return [full[i * B_SHARD : (i + 1) * B_SHARD] for i in range(N_CORES)]


def kernel(x_clinical, x_mRNA, x_miRNA, x_CNV, **run_kwargs):
    shards = {n: _shard(v) for n, v in zip(NAMES, [x_clinical, x_mRNA, x_miRNA, x_CNV])}
    in_maps = [{n: shards[n][i] for n in NAMES} for i in range(N_CORES)]
    res = _run(in_maps, **run_kwargs)
    out = np.concatenate([res.results[i]["out"] for i in range(N_CORES)], axis=0)
    if run_kwargs:
        return out, res
    return out
